# revision 10
# baseline (speedup 1.0000x reference)
"""RQ-VAE (4-stage residual VQ, Sinkhorn assignment) Trainium2 kernel.

Per stage (residual r [Nc,64] per core, codebook e [256,64]):
  u = r @ e.T      split as bf16 hi/lo: u ~= xh@eh + xl@eh + xh@el  (fp32 PSUM)
                   A/B passes use the cheap single term xh@eh (tolerance ok).
  gmax = max(rowsq - 2*umin_n), gmin = min(rowsq - 2*umax_n)  -> AllReduce(max)
      (esq dropped: error <= max||e||^2 ~3.7e-4, well under the ~1.5e-3 budget)
  amp = (gmax-gmin)/2 + 1e-5 ; sc2 = 2/(amp*eps)
  P = exp(sc2*(u - umax_n))  bf16, SBUF-resident [128, 256*256]
  Sinkhorn scalings (middle/esq absorbed into c~, T iters, converged by 2):
    rho = 1/(Ntot*rowsum); colsum = P^T rho (PE matvecs) -> AllReduce(add);
    c = 1/(K*colsum); rowsum = P@c (DVE scalar_tensor_tensor per chunk)
  idx = argmax_k(u + gamma_k), gamma = ln(c~)/sc2 injected via the augmented
    65th contraction row (gamma split hi/lo across the two hi-plane matmuls).
  xq = e[idx] (indirect-DMA gather); r -= xq; losses = 1.25*mean(r_next^2).
"""

from contextlib import ExitStack

import numpy as np

import concourse.bass as bass
import concourse.mybir as mybir
import concourse.tile as tile
from concourse.masks import make_identity

F32 = mybir.dt.float32
BF16 = mybir.dt.bfloat16
U16 = mybir.dt.uint16
I32 = mybir.dt.int32
U32 = mybir.dt.uint32

K = 256          # codebook entries
D = 64           # embedding dim
S = 4            # quantizer stages
EPS = 0.003
T_SINK = 3       # sinkhorn iterations (converged by 2; 3 = margin)


def build_rqvae(tc: tile.TileContext, outs: dict, ins: dict, nchunks: int, ncores: int):
    """Emit the kernel IR. ins/outs are dicts of DRAM APs.

    ins:  x_sh [Nc,64] f32, xTh_sh/xTl_sh [65,Nc] bf16 (row 64 = ones),
          rowsq0 [128,nchunks] f32, cb0..cb3 [256,64] f32, cbT [4,64,256] f32
    outs: r4_out [Nc,64] f32, idx_out [4,128,nchunks] i32, loss_out [128,8] f32
    """
    nc = tc.nc
    NCH = nchunks
    Nc = NCH * 128
    NTOT = Nc * ncores
    rg = [list(range(ncores))]
    GT = 4            # chunks per rT stream tile
    GU = 4            # chunks per update block
    GG = 32           # chunks per gather block

    x_sh, rowsq0 = ins["x_sh"], ins["rowsq0"]
    xTh_sh, xTl_sh = ins["xTh_sh"], ins["xTl_sh"]
    cbs = [ins[f"cb{s}"] for s in range(S)]
    cbT = ins["cbT"]
    r4_out, idx_out, loss_out = outs["r4_out"], outs["idx_out"], outs["loss_out"]

    gg = min(GG, NCH)
    MXT = mybir.AxisListType.X
    OP = mybir.AluOpType
    AF = mybir.ActivationFunctionType

    ctx = ExitStack()
    with ctx:
        sb = ctx.enter_context(tc.tile_pool(name="sb", bufs=1))
        sct = ctx.enter_context(tc.tile_pool(name="sct", bufs=3))     # rT stream tiles
        scr = ctx.enter_context(tc.tile_pool(name="scr", bufs=4))     # score / scratch
        sup = ctx.enter_context(tc.tile_pool(name="sup", bufs=3))     # update blocks
        pmm = ctx.enter_context(tc.tile_pool(name="pmm", bufs=3, space="PSUM"))
        pcs = ctx.enter_context(tc.tile_pool(name="pcs", bufs=1, space="PSUM"))
        pmisc = ctx.enter_context(tc.tile_pool(name="pmisc", bufs=3, space="PSUM"))
        dram = ctx.enter_context(tc.tile_pool(name="dram", bufs=1, space="DRAM"))

        # ---- persistent SBUF state ----
        Pbuf = sb.tile([128, K * NCH], BF16)            # Sinkhorn kernel matrix
        rowsqbuf = sb.tile([128, NCH], F32)
        umaxbuf = sb.tile([128, NCH], F32)
        uminbuf = sb.tile([128, NCH], F32)
        rowsumbuf = sb.tile([128, NCH], F32)
        biasbuf = sb.tile([128, NCH], F32)
        rho32 = sb.tile([128, NCH], F32)
        rho_bf = sb.tile([128, NCH], BF16)
        cb_bc = sb.tile([128, K], BF16)
        idx8buf = sb.tile([128, 8 * NCH], U16)
        idxout_sb = sb.tile([128, NCH], I32)
        losspart = sb.tile([128, 8], F32)
        identity = sb.tile([128, 128], BF16)
        cbh_sb = sb.tile([128, 2 * D], BF16)
        cbl_sb = sb.tile([128, 2 * D], BF16)
        cbf32 = sb.tile([128, 2 * D], F32)
        ones128 = sb.tile([1, 128], F32)
        onesrow_bf = sb.tile([1, 2048], BF16)
        cs_sb = sb.tile([128, 2], F32)
        c32 = sb.tile([128, 2], F32)
        lnc = sb.tile([128, 2], F32)
        gamarr = sb.tile([128, 2], F32)
        crow = sb.tile([1, K], F32)
        gamrow = sb.tile([1, K], F32)
        gamrow_h = sb.tile([1, K], BF16)
        gamrow_l = sb.tile([1, K], BF16)
        pay = sb.tile([1, 2], F32)
        arres = sb.tile([1, 2], F32)
        mmrow = sb.tile([1, 128], F32)
        mmrow2 = sb.tile([1, 128], F32)
        gmaxl = sb.tile([1, 1], F32)
        gminl = sb.tile([1, 1], F32)
        rng1 = sb.tile([1, 1], F32)
        amp = sb.tile([1, 1], F32)
        ampinv = sb.tile([1, 1], F32)
        sc2 = sb.tile([1, 1], F32)
        g2si = sb.tile([1, 1], F32)
        sc2_bc = sb.tile([128, 1], F32)
        negsc2_bc = sb.tile([128, 1], F32)
        g2si_bc = sb.tile([128, 1], F32)
        cbT_sb = sb.tile([64, K * S], F32)              # all 4 stages' codebooksT
        EH = sb.tile([65, K], BF16)                     # [eh; 0 -> gamma_h]
        EH2 = sb.tile([65, K], BF16)                    # [eh; gamma_l]
        EL = sb.tile([65, K], BF16)                     # [el; 0]
        tmpmm = sb.tile([128, NCH], F32)

        # ---- DRAM scratch ----
        r_dram = dram.tile([Nc, D], F32)
        rTh_dram = dram.tile([65, Nc], BF16)
        rTl_dram = dram.tile([65, Nc], BF16)
        shared = "Shared" if ncores > 4 else "Local"

        def ar_pair(shape, nm):
            ci = dram.tile(shape, F32, name=f"ari_{nm}")
            co = dram.tile(shape, F32, addr_space=shared, name=f"aro_{nm}")
            return ci, co

        bounce = dram.tile([1, 256], F32)
        bounce2 = dram.tile([1, 256], F32)

        make_identity(nc, identity[:])
        nc.vector.memset(ones128[:], 1.0)
        nc.vector.memset(onesrow_bf[:], 1.0)
        nc.vector.memset(losspart[:], 0.0)
        nc.sync.dma_start(rowsqbuf[:], rowsq0[:])
        nc.sync.dma_start(cbT_sb[:].rearrange("d (s k) -> d s k", k=K),
                          cbT[:].rearrange("s d k -> d s k"))
        qs = min(2048, Nc)
        for q in range(Nc // qs):
            nc.sync.dma_start(rTh_dram[64:65, q * qs:(q + 1) * qs], onesrow_bf[:, :qs])
            nc.sync.dma_start(rTl_dram[64:65, q * qs:(q + 1) * qs], onesrow_bf[:, :qs])

        def bcast128(dst, src11):
            ps = pmisc.tile([128, 1], F32, name=f"bc_{dst.tensor.name}", tag="misc")
            nc.tensor.matmul(ps[:], lhsT=ones128[:], rhs=src11[:], start=True, stop=True)
            nc.scalar.copy(dst[:], ps[:])

        def flip_to_row(dst_row, src_col2, dram_tile, n):
            """[128, m] (k = h*128 + p) -> [1, n] row via DRAM bounce."""
            m = src_col2.shape[1]
            assert n == 128 * m
            nc.sync.dma_start(
                dram_tile[:, :n].rearrange("o (h p) -> o h p", h=m)
                .rearrange("o h p -> (o p) h"),
                src_col2[:])
            nc.sync.dma_start(dst_row[:, :n], dram_tile[:, :n])

        for s in range(S):
            cbT_s = cbT_sb[:, s * K:(s + 1) * K]
            rTh_src = xTh_sh if s == 0 else rTh_dram[:]
            rTl_src = xTl_sh if s == 0 else rTl_dram[:]
            r_src = x_sh if s == 0 else r_dram[:]

            # stage-local codebook planes for the onehot gather
            nc.sync.dma_start(cbf32[:].rearrange("k (h d) -> k h d", d=D),
                              cbs[s][:].rearrange("(h k) d -> k h d", h=2))
            nc.scalar.copy(cbh_sb[:], cbf32[:])
            nc.vector.tensor_sub(cbl_sb[:], cbf32[:], cbh_sb[:])

            # stage-local E matrices
            nc.scalar.copy(EH[0:64, :], cbT_s)
            nc.vector.tensor_sub(EL[0:64, :], cbT_s, EH[0:64, :])
            nc.vector.tensor_copy(EH2[0:64, :], EH[0:64, :])
            nc.vector.memset(EH[64:65, :], 0.0)
            nc.vector.memset(EL[64:65, :], 0.0)

            def stream_matmul(body, tag):
                for g in range(NCH // GT):
                    rt_t = sct.tile([65, GT * 128], BF16, name=f"rt_{tag}", tag="rt")
                    nc.sync.dma_start(rt_t[:], rTh_src[:, g * GT * 128:(g + 1) * GT * 128])
                    for j in range(GT):
                        c = g * GT + j
                        upsum = pmm.tile([128, K], F32, name=f"u_{tag}", tag="upsum")
                        nc.tensor.matmul(upsum[:], lhsT=rt_t[:, j * 128:(j + 1) * 128],
                                         rhs=EH[:], start=True, stop=True)
                        body(c, upsum)

            # ---------- pass A: u extremes -> amp, sc2 ----------
            def a_body(c, upsum):
                nc.vector.tensor_reduce(umaxbuf[:, c:c + 1], upsum[:], axis=MXT, op=OP.max)
                nc.vector.tensor_reduce(uminbuf[:, c:c + 1], upsum[:], axis=MXT, op=OP.min)
            stream_matmul(a_body, f"a{s}")

            nc.vector.scalar_tensor_tensor(out=tmpmm[:], in0=uminbuf[:], scalar=-2.0,
                                           in1=rowsqbuf[:], op0=OP.mult, op1=OP.add)
            mcol = scr.tile([128, 1], F32, name="mcol", tag="mcol")
            nc.vector.tensor_reduce(mcol[:], tmpmm[:], axis=MXT, op=OP.max)
            flip_to_row(mmrow, mcol, bounce, 128)
            nc.vector.tensor_reduce(gmaxl[:], mmrow[:], axis=MXT, op=OP.max)
            nc.vector.scalar_tensor_tensor(out=tmpmm[:], in0=umaxbuf[:], scalar=-2.0,
                                           in1=rowsqbuf[:], op0=OP.mult, op1=OP.add)
            mcol2 = scr.tile([128, 1], F32, name="mcol2", tag="mcol")
            nc.vector.tensor_reduce(mcol2[:], tmpmm[:], axis=MXT, op=OP.min)
            flip_to_row(mmrow2, mcol2, bounce2, 128)
            nc.vector.tensor_reduce(gminl[:], mmrow2[:], axis=MXT, op=OP.min)
            nc.vector.tensor_copy(pay[:, 0:1], gmaxl[:])
            nc.vector.tensor_scalar_mul(pay[:, 1:2], gminl[:], -1.0)
            mm_in, mm_out = ar_pair([1, 2], f"mm{s}")
            nc.sync.dma_start(mm_in[:], pay[:])
            nc.gpsimd.collective_compute("AllReduce", OP.max, replica_groups=rg,
                                         ins=[mm_in[:]], outs=[mm_out[:]])
            nc.sync.dma_start(arres[:], mm_out[:])
            nc.vector.tensor_reduce(rng1[:], arres[:], axis=MXT, op=OP.add)
            nc.vector.tensor_scalar(out=amp[:], in0=rng1[:], scalar1=0.5, scalar2=1e-5,
                                    op0=OP.mult, op1=OP.add)
            nc.vector.reciprocal(ampinv[:], amp[:])
            nc.vector.tensor_scalar_mul(sc2[:], ampinv[:], 2.0 / EPS)
            nc.vector.reciprocal(g2si[:], sc2[:])
            bcast128(sc2_bc, sc2)
            bcast128(g2si_bc, g2si)
            nc.vector.tensor_scalar_mul(negsc2_bc[:], sc2_bc[:], -1.0)
            nc.vector.tensor_scalar(out=biasbuf[:], in0=umaxbuf[:], scalar1=negsc2_bc[:],
                                    scalar2=None, op0=OP.mult)

            # ---------- pass B: P = exp(sc2*(u - umax)), rowsum0 ----------
            def b_body(c, upsum):
                nc.scalar.activation(out=Pbuf[:, c * K:(c + 1) * K], in_=upsum[:],
                                     func=AF.Exp, bias=biasbuf[:, c:c + 1],
                                     scale=sc2_bc[:], accum_out=rowsumbuf[:, c:c + 1])
            stream_matmul(b_body, f"b{s}")

            # ---------- sinkhorn iterations ----------
            for t in range(T_SINK):
                nc.vector.reciprocal_approx_fast(rho32[:], rowsumbuf[:])
                nc.scalar.activation(out=rho_bf[:], in_=rho32[:], func=AF.Copy,
                                     scale=1.0 / NTOT)
                csp = [pcs.tile([128, 1], F32, name=f"csp{h}_{s}_{t}", tag=f"csp{h}")
                       for h in range(2)]
                for c in range(NCH):
                    for h in range(2):
                        nc.tensor.matmul(
                            csp[h][:], lhsT=Pbuf[:, c * K + h * 128: c * K + h * 128 + 128],
                            rhs=rho_bf[:, c:c + 1], start=(c == 0), stop=(c == NCH - 1))
                for h in range(2):
                    nc.scalar.activation(out=cs_sb[:, h:h + 1], in_=csp[h][:],
                                         func=AF.Copy, scale=float(K))
                cc_in, cc_out = ar_pair([128, 2], f"cc{s}_{t}")
                nc.sync.dma_start(cc_in[:], cs_sb[:])
                nc.gpsimd.collective_compute("AllReduce", OP.add, replica_groups=rg,
                                             ins=[cc_in[:]], outs=[cc_out[:]])
                nc.sync.dma_start(c32[:], cc_out[:])
                nc.vector.reciprocal_approx_fast(c32[:], c32[:])
                if t < T_SINK - 1:
                    flip_to_row(crow, c32, bounce, K)
                    cbps = pmisc.tile([128, K], F32, name="cbps", tag="misc")
                    nc.tensor.matmul(cbps[:], lhsT=ones128[:], rhs=crow[:],
                                     start=True, stop=True)
                    nc.scalar.copy(cb_bc[:], cbps[:])
                    for c in range(NCH):
                        scb = scr.tile([128, K], BF16, name="scb", tag="scb")
                        nc.vector.scalar_tensor_tensor(
                            out=scb[:], in0=Pbuf[:, c * K:(c + 1) * K], scalar=1.0,
                            in1=cb_bc[:], op0=OP.mult, op1=OP.mult,
                            accum_out=rowsumbuf[:, c:c + 1])

            # ---------- pass C: argmax(u + gamma) ----------
            nc.scalar.activation(out=lnc[:], in_=c32[:], func=AF.Ln)
            nc.vector.tensor_scalar(out=gamarr[:], in0=lnc[:], scalar1=g2si_bc[:],
                                    scalar2=None, op0=OP.mult)
            flip_to_row(gamrow, gamarr, bounce2, K)
            nc.scalar.copy(gamrow_h[:], gamrow[:])
            nc.vector.tensor_sub(gamrow_l[:], gamrow[:], gamrow_h[:])
            nc.vector.tensor_copy(EH[64:65, :], gamrow_h[:])
            nc.vector.tensor_copy(EH2[64:65, :], gamrow_l[:])

            last = s == S - 1
            r_dst = r4_out if last else r_dram[:]
            for g in range(NCH // GT):
                rth_t = sct.tile([65, GT * 128], BF16, name=f"rth_c{s}", tag="rt")
                rtl_t = sct.tile([65, GT * 128], BF16, name=f"rtl_c{s}", tag="rtl")
                nc.sync.dma_start(rth_t[:], rTh_src[:, g * GT * 128:(g + 1) * GT * 128])
                nc.sync.dma_start(rtl_t[:], rTl_src[:, g * GT * 128:(g + 1) * GT * 128])
                rblk = sup.tile([128, GT * D], F32, name="rblk", tag="rblk")
                nc.sync.dma_start(
                    rblk[:].rearrange("p (c d) -> p c d", d=D),
                    r_src[g * GT * 128:(g + 1) * GT * 128, :]
                    .rearrange("(c p) d -> p c d", p=128))
                rnew = sup.tile([128, GT * D], F32, name="rnew", tag="rnew")
                for j in range(GT):
                    c = g * GT + j
                    upsum = pmm.tile([128, K], F32, name=f"u_c{s}", tag="upsum")
                    lh = rth_t[:, j * 128:(j + 1) * 128]
                    ll = rtl_t[:, j * 128:(j + 1) * 128]
                    nc.tensor.matmul(upsum[:], lhsT=lh, rhs=EH[:], start=True, stop=False)
                    nc.tensor.matmul(upsum[:], lhsT=lh, rhs=EL[:], start=False, stop=False)
                    nc.tensor.matmul(upsum[:], lhsT=ll, rhs=EH2[:], start=False, stop=True)
                    score = scr.tile([128, K], F32, name="score", tag="score")
                    nc.scalar.copy(score[:], upsum[:])
                    m8 = scr.tile([128, 8], F32, name="m8", tag="m8")
                    nc.vector.max(m8[:], score[:])
                    nc.vector.max_index(idx8buf[:, c * 8:(c + 1) * 8], m8[:], score[:])
                    # onehot gather: xq = onehot^T-contracted codebook (hi+lo planes)
                    oh = scr.tile([128, K], BF16, name="oh", tag="oh")
                    nc.vector.tensor_scalar(out=oh[:], in0=score[:], scalar1=m8[:, 0:1],
                                            scalar2=None, op0=OP.is_ge)
                    ohT_sb = scr.tile([128, K], BF16, name="ohT_sb", tag="ohT_sb")
                    for h in range(2):
                        tpo = pmisc.tile([128, 128], BF16, name="tpo", tag="misc")
                        nc.tensor.transpose(tpo[:], in_=oh[:, h * 128:(h + 1) * 128],
                                            identity=identity[:])
                        nc.scalar.copy(ohT_sb[:, h * 128:(h + 1) * 128], tpo[:])
                    xqps = pmisc.tile([128, D], F32, name="xqps", tag="misc")
                    for h in range(2):
                        nc.tensor.matmul(xqps[:], lhsT=ohT_sb[:, h * 128:(h + 1) * 128],
                                         rhs=cbh_sb[:, h * D:(h + 1) * D],
                                         start=(h == 0), stop=False)
                        nc.tensor.matmul(xqps[:], lhsT=ohT_sb[:, h * 128:(h + 1) * 128],
                                         rhs=cbl_sb[:, h * D:(h + 1) * D],
                                         start=False, stop=(h == 1))
                    nc.vector.tensor_sub(rnew[:, j * D:(j + 1) * D],
                                         rblk[:, j * D:(j + 1) * D], xqps[:])
                sqb = sup.tile([128, GT * D], F32, name="sqb", tag="sqb")
                nc.scalar.square(sqb[:], rnew[:])
                nc.vector.tensor_reduce(
                    rowsqbuf[:, g * GT:(g + 1) * GT],
                    sqb[:].rearrange("p (c d) -> p c d", d=D), axis=MXT, op=OP.add)
                nc.sync.dma_start(
                    r_dst[g * GT * 128:(g + 1) * GT * 128, :]
                    .rearrange("(c p) d -> p c d", p=128),
                    rnew[:].rearrange("p (c d) -> p c d", d=D))
                if not last:
                    xhb = sup.tile([128, GT * D], BF16, name="xhb", tag="xhb")
                    nc.scalar.copy(xhb[:], rnew[:])
                    xlb = sup.tile([128, GT * D], BF16, name="xlb", tag="xlb")
                    nc.vector.tensor_sub(xlb[:], rnew[:], xhb[:])
                    rth_st = sup.tile([64, GT * 128], BF16, name="rth_st", tag="rth_st")
                    rtl_st = sup.tile([64, GT * 128], BF16, name="rtl_st", tag="rtl_st")
                    for j in range(GT):
                        tps = pmisc.tile([64, 128], BF16, name="tps", tag="misc")
                        nc.tensor.transpose(tps[:], in_=xhb[:, j * D:(j + 1) * D],
                                            identity=identity[:])
                        nc.scalar.copy(rth_st[:, j * 128:(j + 1) * 128], tps[:])
                        tps2 = pmisc.tile([64, 128], BF16, name="tps2", tag="misc")
                        nc.tensor.transpose(tps2[:], in_=xlb[:, j * D:(j + 1) * D],
                                            identity=identity[:])
                        nc.scalar.copy(rtl_st[:, j * 128:(j + 1) * 128], tps2[:])
                    nc.sync.dma_start(
                        rTh_dram[0:64, g * GT * 128:(g + 1) * GT * 128], rth_st[:])
                    nc.sync.dma_start(
                        rTl_dram[0:64, g * GT * 128:(g + 1) * GT * 128], rtl_st[:])

            nc.vector.tensor_copy(
                idxout_sb[:],
                idx8buf[:].rearrange("p (c e) -> p c e", e=8)[:, :, 0:1]
                .rearrange("p c e -> p (c e)"))
            nc.sync.dma_start(idx_out[s, :, :], idxout_sb[:])

            lpc = scr.tile([128, 1], F32, name="lpc", tag="mcol")
            nc.vector.tensor_reduce(lpc[:], rowsqbuf[:], axis=MXT, op=OP.add)
            nc.vector.tensor_copy(losspart[:, s:s + 1], lpc[:])

        nc.sync.dma_start(loss_out[:], losspart[:])


def emulate_numpy(x_sh_list, codebooks, nchunks, T=T_SINK):
    """Golden emulation of the device algorithm across cores."""
    import ml_dtypes
    bf16 = ml_dtypes.bfloat16
    ncores = len(x_sh_list)
    Nc = nchunks * 128
    NTOT = Nc * ncores

    rs = [x.copy() for x in x_sh_list]
    idx_all = [np.zeros((Nc, S), np.int64) for _ in range(ncores)]
    loss_sums = np.zeros(S, np.float64)
    for s in range(S):
        cbk = codebooks[s].astype(np.float32)
        eh = cbk.astype(bf16)
        el = (cbk - eh.astype(np.float32)).astype(bf16)
        eh32, el32 = eh.astype(np.float32), el.astype(np.float32)
        us, xs_h, xs_l = [], [], []
        gmax, gmin = -np.inf, np.inf
        for r in rs:
            xh = r.astype(bf16)
            xl = (r - xh.astype(np.float32)).astype(bf16)
            xh32, xl32 = xh.astype(np.float32), xl.astype(np.float32)
            xs_h.append(xh32); xs_l.append(xl32)
            u = (xh32 @ eh32.T).astype(np.float32)
            rowsq = (r * r).sum(1).astype(np.float32)
            us.append(u)
            gmax = max(gmax, (rowsq - 2 * u.min(1)).max())
            gmin = min(gmin, (rowsq - 2 * u.max(1)).min())
        ampv = np.float32((gmax - gmin) / 2.0 + 1e-5)
        sc2v = np.float32(2.0 / (ampv * EPS))
        Ps = [np.exp((sc2v * (u - u.max(1, keepdims=True))).astype(np.float32))
              .astype(bf16).astype(np.float32) for u in us]
        rowsums = [P.sum(1, dtype=np.float32) for P in Ps]
        c = None
        for t in range(T):
            colsum = np.zeros(K, np.float32)
            for P, rowsum in zip(Ps, rowsums):
                rho = ((1.0 / rowsum).astype(np.float32) / np.float32(NTOT)) \
                    .astype(bf16).astype(np.float32)
                colsum += (P.T @ rho).astype(np.float32) * np.float32(K)
            c = (1.0 / colsum).astype(np.float32)
            if t < T - 1:
                cbc = c.astype(bf16).astype(np.float32)
                rowsums = [(P * cbc[None, :]).sum(1, dtype=np.float32) for P in Ps]
        gam = (np.log(c) / sc2v).astype(np.float32)
        gh = gam.astype(bf16).astype(np.float32)
        gl = (gam - gh).astype(bf16).astype(np.float32)
        for i in range(ncores):
            score = (((xs_h[i] @ eh32.T + gh[None, :])
                      + (xs_h[i] @ el32.T))
                     + (xs_l[i] @ eh32.T + gl[None, :])).astype(np.float32)
            idx = score.argmax(1)
            idx_all[i][:, s] = idx
            rs[i] = (rs[i] - cbk[idx]).astype(np.float32)
            loss_sums[s] += float((rs[i].astype(np.float64) ** 2).sum())
    return rs, idx_all, loss_sums


# ======================================================================
# Host driver: shard over 8 cores, compile once, run, unshard.
# ======================================================================

NCORES = 8
NCHUNKS = 256            # 256 chunks * 128 rows = 32768 rows per core
NC_ROWS = NCHUNKS * 128
N_FULL = NC_ROWS * NCORES

_compiled = None


def _get_compiled():
    global _compiled
    if _compiled is not None:
        return _compiled
    import concourse.bacc as bacc

    nc = bacc.Bacc("TRN2", target_bir_lowering=False, debug=False,
                   enable_asserts=False, num_devices=NCORES)
    ins = {
        "x_sh": nc.dram_tensor("x_sh", [NC_ROWS, D], F32, kind="ExternalInput").ap(),
        "xTh_sh": nc.dram_tensor("xTh_sh", [65, NC_ROWS], BF16, kind="ExternalInput").ap(),
        "xTl_sh": nc.dram_tensor("xTl_sh", [65, NC_ROWS], BF16, kind="ExternalInput").ap(),
        "rowsq0": nc.dram_tensor("rowsq0", [128, NCHUNKS], F32, kind="ExternalInput").ap(),
        "cbT": nc.dram_tensor("cbT", [S, D, K], F32, kind="ExternalInput").ap(),
    }
    for s in range(S):
        ins[f"cb{s}"] = nc.dram_tensor(f"cb{s}", [K, D], F32, kind="ExternalInput").ap()
    outs = {
        "r4_out": nc.dram_tensor("r4_out", [NC_ROWS, D], F32, kind="ExternalOutput").ap(),
        "idx_out": nc.dram_tensor("idx_out", [S, 128, NCHUNKS], I32, kind="ExternalOutput").ap(),
        "loss_out": nc.dram_tensor("loss_out", [128, 8], F32, kind="ExternalOutput").ap(),
    }
    with tile.TileContext(nc) as tc:
        build_rqvae(tc, outs, ins, NCHUNKS, NCORES)
    nc.compile()
    _compiled = nc
    return nc


def _make_in_maps(x, codebooks):
    import ml_dtypes
    bf16 = ml_dtypes.bfloat16
    x = np.ascontiguousarray(x, dtype=np.float32)
    cb = np.ascontiguousarray(codebooks, dtype=np.float32)
    cbT = np.ascontiguousarray(cb.transpose(0, 2, 1))
    ones = np.ones((1, NC_ROWS), bf16)
    in_maps = []
    for i in range(NCORES):
        xs = np.ascontiguousarray(x[i * NC_ROWS:(i + 1) * NC_ROWS])
        xh = xs.astype(bf16)
        xl = (xs - xh.astype(np.float32)).astype(bf16)
        m = {
            "x_sh": xs,
            "xTh_sh": np.ascontiguousarray(np.vstack([xh.T, ones])),
            "xTl_sh": np.ascontiguousarray(np.vstack([xl.T, ones])),
            "rowsq0": np.ascontiguousarray(
                (xs.astype(np.float32) ** 2).sum(1, dtype=np.float32)
                .reshape(NCHUNKS, 128).T),
            "cbT": cbT,
        }
        for s in range(S):
            m[f"cb{s}"] = np.ascontiguousarray(cb[s])
        in_maps.append(m)
    return in_maps


def run_on_hw(x, codebooks, trace=False):
    from concourse import bass_utils
    nc = _get_compiled()
    in_maps = _make_in_maps(x, codebooks)
    res = bass_utils.run_bass_kernel_spmd(
        nc, in_maps, core_ids=list(range(NCORES)), trace=trace)
    return res


def _unshard(x, results):
    xq = np.empty((N_FULL, D), np.float32)
    idx = np.empty((N_FULL, S), np.int64)
    loss_sums = np.zeros(S, np.float64)
    for i, out in enumerate(results):
        r4 = out["r4_out"]
        xq[i * NC_ROWS:(i + 1) * NC_ROWS] = x[i * NC_ROWS:(i + 1) * NC_ROWS] - r4
        idxd = out["idx_out"]                      # [S,128,NCH]
        for s in range(S):
            idx[i * NC_ROWS:(i + 1) * NC_ROWS, s] = \
                idxd[s].astype(np.int64).T.reshape(-1)
        loss_sums += out["loss_out"][:, :S].sum(axis=0, dtype=np.float64)
    loss = np.float32(np.mean(1.25 * loss_sums / (N_FULL * D)))
    return xq, loss, idx


def kernel(x, codebooks):
    x = np.asarray(x, dtype=np.float32)
    codebooks = np.asarray(codebooks, dtype=np.float32)
    assert x.shape == (N_FULL, D) and codebooks.shape == (S, K, D)
    res = run_on_hw(x, codebooks, trace=False)
    return _unshard(x, res.results)
